# revision 21
# baseline (speedup 1.0000x reference)
"""Mixtral-style MoE (top-2 of 8 experts) on 8 TRN2 NeuronCores.

Strategy (expert-parallel, matching TENSOR_EXPERT_PARALLEL):
  - Host: router (logits -> softmax -> top-2 -> normalized weights), then
    shard: core e receives the tokens routed to expert e (gathered and
    pre-transposed to [H, C]) plus expert e's w1/w3/w2 (bf16, pre-packed
    into PE-friendly [128 x free] tiles).
  - Device (SPMD, identical program on 8 cores): h1T = w1 @ xeT,
    h3T = w3 @ xeT, gT = silu(h1T) * h3T (bf16), then the down-proj in
    output-transposed orientation: outT[h, :] = sum_f w2T-tile @ gT
    (tokens stay on the moving axis, so no padded-partition waste on
    the partial token chunk).  Pure GEMM pipeline; all DMAs linear.
  - Host: scatter-add each core's [H, count_e] contribution (scaled by
    the routing weight, applied host-side) into the [T, H] output.

Compute in bf16 (fp32 PSUM accumulation) keeps the TensorEngine at its
78.6 TF/s peak; fp8 DoubleRow would be ~1.8x faster but its ~3-6%
quantization error blows the 2e-2 correctness budget (measured).
Sparse routing means each core does C = max expert load (~1071)
token-columns instead of all 4096.

PE-time floor at C=1071: phase A 28*2*8*C = 200us + phase B 8*28*C =
100us = 300us @2.4GHz.  Engine init (~6.6us) and first DMA bytes
(~8.3us) are fixed NEFF costs; warmup matmuls bridge them while the
HAM clock ramps.  fp0+fp1 run hk-outer *interleaved* with the token
range split {ci0,ci1} then {ci2} (exactly 8 PSUM banks), stretching
the xe consumption window so three ~130GB/s DMA queues deliver every
chunk just in time -- no stalls, no half-clock dip.  Phase B stages
each 128-row H chunk in a full-width SBUF tile and ships it as one
large-packet DMA, alternating queues, so the output drain collapses
to the final piece plus teardown.
"""

import numpy as np
import ml_dtypes

B, S, H, F, E, TOP_K = 2, 2048, 1024, 3584, 8, 2
N_CORES = 8
P = 128
HK = H // P   # 8 contraction chunks for up-proj
FP = F // P   # 28 partition chunks of the FFN dim
HO = H // P   # 8 output-row chunks of H for the down-proj

BF16 = ml_dtypes.bfloat16

_BUILD_CACHE = {}
LAST_EXEC_TIME_NS = None


def _ensure_axon_hooks_stub():
    """bass_utils imports antenv.axon_hooks when BASS_TRACE is set; the
    agent image lacks it.  Register a None-hook stub so a stray
    BASS_TRACE env var degrades to an untraced run instead of crashing.
    """
    import sys, types

    try:
        import antenv.axon_hooks  # noqa: F401
        return
    except ImportError:
        pass
    mod = types.ModuleType("antenv.axon_hooks")
    mod._hook = None
    mod.set_axon_ntff_profile_hook = lambda h: setattr(mod, "_hook", h)
    mod.get_axon_ntff_profile_hook = lambda: mod._hook
    sys.modules["antenv.axon_hooks"] = mod
    try:
        import antenv

        antenv.axon_hooks = mod
    except ImportError:
        pass


def _chunks(total, maxc):
    """Split `total` into equal-ish chunks <= maxc (PSUM free-dim cap)."""
    n = -(-total // maxc)
    base, rem = divmod(total, n)
    sizes = [base + (1 if i < rem else 0) for i in range(n)]
    out, off = [], 0
    for c in sizes:
        out.append((off, c))
        off += c
    return out


def _build(C):
    """Build + compile the SPMD Bass program for token capacity C."""
    import concourse.bacc as bacc
    import concourse.mybir as mybir
    from concourse.tile import TileContext

    bf = mybir.dt.bfloat16
    f32 = mybir.dt.float32

    nc = bacc.Bacc("TRN2", target_bir_lowering=False, debug=False,
                   num_devices=N_CORES)
    # xe is packed [P, HK*C]: per-partition rows hold all HK contraction
    # chunks contiguously, so a multi-chunk column range is ONE wide DMA
    # (queues are descriptor-rate limited: any [128, w] piece costs
    # ~2.1us regardless of w, so fewer/wider pieces win).
    xe = nc.dram_tensor("xe", [P, HK * C], bf, kind="ExternalInput")
    w1p = nc.dram_tensor("w1p", [FP, P, H], bf, kind="ExternalInput")
    w3p = nc.dram_tensor("w3p", [FP, P, H], bf, kind="ExternalInput")
    w2p = nc.dram_tensor("w2p", [FP, P, H], bf, kind="ExternalInput")
    outT = nc.dram_tensor("outT", [HO, P, C], bf, kind="ExternalOutput")

    cn_chunks = _chunks(C, 512)
    NCI = len(cn_chunks)
    silu = mybir.ActivationFunctionType.Silu
    copy = mybir.ActivationFunctionType.Copy

    with TileContext(nc) as tc:
        with (
            tc.tile_pool(name="persist", bufs=1) as persist,
            tc.tile_pool(name="wload", bufs=3) as wload,
            tc.tile_pool(name="gpool", bufs=1) as gpool,
            tc.tile_pool(name="evac", bufs=4) as evac,
            tc.tile_pool(name="ost", bufs=3) as ost,
            tc.tile_pool(name="psum", bufs=4, space="PSUM") as psum,
        ):
            # HAM warmup: the PE clock-gate needs ~3.4us of sustained
            # activity to lift 1.2 -> 2.4 GHz, and the first DMA bytes
            # only land ~8.3us in (engine init + queue spin-up).  Dummy
            # matmuls bridge the gap; the memset runs on GpSimd.
            warm = persist.tile([P, 512], bf, tag="warm", name="warm")
            nc.gpsimd.memset(warm[:], 0.0)
            # 18 dummies (~420-760ns each -- same-bank groups don't
            # pipeline) bridge engine-start (~7.7us) to ~15.5us: the
            # startup set (xe 2.2MB + w1t0/w3t0) needs ~8us of ~400GB/s
            # aggregate DMA, so real chains can't run sooner anyway.
            # The PE never idles pre-ramp, keeping the HAM gate up.
            wps = psum.tile([P, 512], f32, tag="ps1", name="wps")
            for i in range(18):
                nc.tensor.matmul(wps[:], warm[:, 0:P], warm[:],
                                 start=True, stop=True)

            # Startup DMA schedule, ~133GB/s per queue (3-way HBM split).
            # The fp0 hk-outer chain starts as soon as w1t0/w3t0/xe[hk0]
            # land (~10.3us, still on the ramping clock) and consumes one
            # hk chunk per ~1us; each later piece lands just ahead of use.
            xet = persist.tile([P, HK * C], bf, tag="xe", name="xet")

            w1t0 = wload.tile([P, H], bf, tag="w1")
            nc.sync.dma_start(out=w1t0[:], in_=w1p[0])
            w3t0 = wload.tile([P, H], bf, tag="w3")
            nc.scalar.dma_start(out=w3t0[:], in_=w3p[0])
            nc.gpsimd.dma_start(out=xet[:, 0:2 * C], in_=xe[:, 0:2 * C])

            nc.sync.dma_start(out=xet[:, 2 * C:4 * C], in_=xe[:, 2 * C:4 * C])
            nc.scalar.dma_start(out=xet[:, 4 * C:6 * C],
                                in_=xe[:, 4 * C:6 * C])
            nc.gpsimd.dma_start(out=xet[:, 6 * C:8 * C],
                                in_=xe[:, 6 * C:8 * C])

            w1t1 = wload.tile([P, H], bf, tag="w1")
            nc.sync.dma_start(out=w1t1[:], in_=w1p[1])
            w3t1 = wload.tile([P, H], bf, tag="w3")
            nc.scalar.dma_start(out=w3t1[:], in_=w3p[1])

            # w2 residents stream on the gpsimd queue during phase A.
            w2t = []
            for fp in range(FP):
                t = persist.tile([P, H], bf, tag=f"w2_{fp}", name=f"w2_{fp}")
                nc.gpsimd.dma_start(out=t[:], in_=w2p[fp])
                w2t.append(t)

            gt = [gpool.tile([P, C], bf, tag=f"g{fp}", name=f"g{fp}")
                  for fp in range(FP)]

            # Phase A: h1T/h3T = w1/w3 @ xeT per 128-row chunk of F,
            # fused SwiGLU into gT (bf16).
            for fp in range(FP):
                if fp < 2:
                    # hk-outer: each matmul chain consumes xe[hk] as it
                    # lands instead of stalling on the whole activation
                    # load before the first instruction.  Two chains
                    # (~15us PE) cover the startup DMA window.
                    w1t, w3t = (w1t0, w3t0) if fp == 0 else (w1t1, w3t1)
                    pss = {}
                    for mat in (1, 3):
                        for ci in range(NCI):
                            pss[(mat, ci)] = psum.tile(
                                [P, 512], f32, tag=f"ps{mat}",
                                name=f"ps{mat}_c{ci}_f{fp}",
                            )
                    for hk in range(HK):
                        for mat, wt in ((1, w1t), (3, w3t)):
                            for ci, (coff, csz) in enumerate(cn_chunks):
                                nc.tensor.matmul(
                                    pss[(mat, ci)][:, :csz],
                                    wt[:, hk * P:(hk + 1) * P],
                                    xet[:, hk * C + coff:hk * C + coff + csz],
                                    start=(hk == 0), stop=(hk == HK - 1),
                                )
                    for ci, (coff, csz) in enumerate(cn_chunks):
                        sil = evac.tile([P, 512], f32, tag="sil",
                                        name=f"sil_f{fp}_{ci}")
                        nc.scalar.activation(
                            sil[:, :csz], pss[(1, ci)][:, :csz], silu)
                        nc.vector.tensor_mul(
                            gt[fp][:, coff:coff + csz], sil[:, :csz],
                            pss[(3, ci)][:, :csz],
                        )
                    continue
                w1t = wload.tile([P, H], bf, tag="w1")
                nc.sync.dma_start(out=w1t[:], in_=w1p[fp])
                w3t = wload.tile([P, H], bf, tag="w3")
                nc.scalar.dma_start(out=w3t[:], in_=w3p[fp])
                for (coff, csz) in cn_chunks:
                    ps1 = psum.tile([P, 512], f32, tag="ps1")
                    ps3 = psum.tile([P, 512], f32, tag="ps3")
                    # interleave the two chains mm-by-mm: half the chain
                    # boundaries, and ldweights always has a full matmul
                    # of slack to hide under
                    for hk in range(HK):
                        nc.tensor.matmul(
                            ps1[:, :csz],
                            w1t[:, hk * P:(hk + 1) * P],
                            xet[:, hk * C + coff:hk * C + coff + csz],
                            start=(hk == 0), stop=(hk == HK - 1),
                        )
                        nc.tensor.matmul(
                            ps3[:, :csz],
                            w3t[:, hk * P:(hk + 1) * P],
                            xet[:, hk * C + coff:hk * C + coff + csz],
                            start=(hk == 0), stop=(hk == HK - 1),
                        )
                    sil = evac.tile([P, 512], f32, tag="sil")
                    nc.scalar.activation(sil[:, :csz], ps1[:, :csz], silu)
                    nc.vector.tensor_mul(
                        gt[fp][:, coff:coff + csz], sil[:, :csz], ps3[:, :csz]
                    )

            # Phase B: outT[h] chunk [128 H-rows, csz tokens] =
            # sum_fp w2T-tile[fp,h] @ gT[fp].  Tokens ride the moving
            # axis, so the partial token chunk costs only its true
            # column count.  Each h stages into one full-width tile and
            # ships as a single large-packet DMA; routing weights are
            # applied host-side.  Shares the phase-A PSUM pool (no
            # pool-transition barrier).
            for h in range(HO):
                oh = ost.tile([P, C], bf, tag="o", name=f"o{h}")
                last_h = h == HO - 1
                # interleave chunk chains in pairs: consecutive matmuls
                # share the same stationary w2 tile and chain boundaries
                # halve
                groups = [list(range(NCI))[i:i + 2]
                          for i in range(0, NCI, 2)]
                for gi, grp in enumerate(groups):
                    pbs = []
                    for j, ci in enumerate(grp):
                        pbs.append(psum.tile(
                            [P, 512], f32, tag="ps1" if j % 2 == 0
                            else "ps3", name=f"pb_h{h}_c{ci}"))
                    for fp in range(FP):
                        for j, ci in enumerate(grp):
                            coff, csz = cn_chunks[ci]
                            nc.tensor.matmul(
                                pbs[j][:, :csz],
                                w2t[fp][:, h * P:(h + 1) * P],
                                gt[fp][:, coff:coff + csz],
                                start=(fp == 0), stop=(fp == FP - 1),
                            )
                    for j, ci in enumerate(grp):
                        coff, csz = cn_chunks[ci]
                        nc.scalar.activation(oh[:, coff:coff + csz],
                                             pbs[j][:, :csz], copy)
                        if last_h:
                            # ship the final h per-chunk as each evicts,
                            # so only the smallest piece drains after the
                            # last matmul (a [128,w] DMA costs ~2.1us for
                            # ANY w -- never split; overlap instead)
                            e = nc.sync if ci % 2 == 0 else nc.scalar
                            e.dma_start(out=outT[h][:, coff:coff + csz],
                                        in_=oh[:, coff:coff + csz])
                if not last_h:
                    e = nc.sync if h % 2 == 0 else nc.scalar
                    e.dma_start(out=outT[h], in_=oh[:])

    nc.compile()
    return nc


def kernel(hidden_states, gate_w, w1, w2, w3, _trace=False):
    global LAST_EXEC_TIME_NS
    _ensure_axon_hooks_stub()
    from concourse.bass_utils import run_bass_kernel_spmd

    x = np.asarray(hidden_states, dtype=np.float32).reshape(-1, H)
    gate_w = np.asarray(gate_w, dtype=np.float32)
    w1 = np.asarray(w1, dtype=np.float32)
    w2 = np.asarray(w2, dtype=np.float32)
    w3 = np.asarray(w3, dtype=np.float32)
    T = x.shape[0]

    # Router (f32, same math as the module): softmax over experts, top-2,
    # renormalized weights.
    logits = x @ gate_w.T
    p = np.exp(logits - logits.max(-1, keepdims=True))
    p /= p.sum(-1, keepdims=True)
    sel = np.argpartition(-p, TOP_K - 1, axis=-1)[:, :TOP_K]
    rw = np.take_along_axis(p, sel, axis=-1)
    rw = rw / rw.sum(-1, keepdims=True)

    idx_e, cv_e = [], []
    for e in range(E):
        hit = sel == e                      # [T, K]
        idx = np.nonzero(hit.any(axis=1))[0]
        w = np.where(hit[idx, 0], rw[idx, 0], rw[idx, 1])
        idx_e.append(idx)
        cv_e.append(w.astype(np.float32))

    # SBUF budget (xe + gT residents) caps the per-run token capacity.
    # Actual data peaks at cmax ~1071; the segment loop only engages for
    # pathologically imbalanced routing.
    CMAX_HW = 1344
    cmax = max(len(i) for i in idx_e)
    n_seg = max(1, -(-cmax // CMAX_HW))
    seg_idx = [np.array_split(idx_e[e], n_seg) for e in range(E)]
    seg_cv = [np.array_split(cv_e[e], n_seg) for e in range(E)]
    C = max(512, max(len(s) for parts in seg_idx for s in parts))

    if C not in _BUILD_CACHE:
        _BUILD_CACHE[C] = _build(C)
    nc = _BUILD_CACHE[C]

    x_bf = x.astype(BF16)
    w_packed = []
    for e in range(E):
        w1pk = np.ascontiguousarray(
            w1[e].astype(BF16).reshape(FP, P, HK, P).transpose(0, 3, 2, 1)
        ).reshape(FP, P, H)
        w3pk = np.ascontiguousarray(
            w3[e].astype(BF16).reshape(FP, P, HK, P).transpose(0, 3, 2, 1)
        ).reshape(FP, P, H)
        w2pk = np.ascontiguousarray(w2[e].T.astype(BF16)).reshape(FP, P, H)
        w_packed.append((w1pk, w3pk, w2pk))

    out = np.zeros((T, H), dtype=np.float32)
    LAST_EXEC_TIME_NS = None
    for seg in range(n_seg):
        in_maps = []
        for e in range(E):
            idx = seg_idx[e][seg]
            n = len(idx)
            xeT = np.zeros((H, C), dtype=BF16)
            xeT[:, :n] = x_bf[idx].T
            w1pk, w3pk, w2pk = w_packed[e]
            # pack [P, HK*C]: partition-major rows holding all HK chunks
            xpk = np.ascontiguousarray(
                xeT.reshape(HK, P, C).transpose(1, 0, 2).reshape(P, HK * C))
            in_maps.append({
                "xe": xpk,
                "w1p": w1pk,
                "w3p": w3pk,
                "w2p": w2pk,
            })
        res = run_bass_kernel_spmd(
            nc, in_maps, core_ids=list(range(N_CORES)), trace=_trace
        )
        if res.exec_time_ns is not None:
            LAST_EXEC_TIME_NS = (LAST_EXEC_TIME_NS or 0) + res.exec_time_ns
        for e in range(E):
            idx = seg_idx[e][seg]
            n = len(idx)
            if n:
                oT = np.asarray(res.results[e]["outT"],
                                dtype=np.float32).reshape(H, C)
                out[idx] += oT[:, :n].T * seg_cv[e][seg][:, None]
    return out.reshape(B, S, H)


# revision 22
# speedup vs baseline: 1.0071x; 1.0071x over previous
"""Mixtral-style MoE (top-2 of 8 experts) on 8 TRN2 NeuronCores.

Strategy (expert-parallel, matching TENSOR_EXPERT_PARALLEL):
  - Host: router (logits -> softmax -> top-2 -> normalized weights), then
    shard: core e receives the tokens routed to expert e (gathered and
    pre-transposed to [H, C]) plus expert e's w1/w3/w2 (bf16, pre-packed
    into PE-friendly [128 x free] tiles).
  - Device (SPMD, identical program on 8 cores): h1T = w1 @ xeT,
    h3T = w3 @ xeT, gT = silu(h1T) * h3T (bf16), then the down-proj in
    output-transposed orientation: outT[h, :] = sum_f w2T-tile @ gT
    (tokens stay on the moving axis, so no padded-partition waste on
    the partial token chunk).  Pure GEMM pipeline; all DMAs linear.
  - Host: scatter-add each core's [H, count_e] contribution (scaled by
    the routing weight, applied host-side) into the [T, H] output.

Compute in bf16 (fp32 PSUM accumulation) keeps the TensorEngine at its
78.6 TF/s peak; fp8 DoubleRow would be ~1.8x faster but its ~3-6%
quantization error blows the 2e-2 correctness budget (measured).
Sparse routing means each core does C = max expert load (~1071)
token-columns instead of all 4096.

PE-time floor at C=1071: phase A 28*2*8*C = 200us + phase B 8*28*C =
100us = 300us @2.4GHz.  Engine init (~6.6us) and first DMA bytes
(~8.3us) are fixed NEFF costs; warmup matmuls bridge them while the
HAM clock ramps.  fp0+fp1 run hk-outer *interleaved* with the token
range split {ci0,ci1} then {ci2} (exactly 8 PSUM banks), stretching
the xe consumption window so three ~130GB/s DMA queues deliver every
chunk just in time -- no stalls, no half-clock dip.  Phase B stages
each 128-row H chunk in a full-width SBUF tile and ships it as one
large-packet DMA, alternating queues, so the output drain collapses
to the final piece plus teardown.
"""

import numpy as np
import ml_dtypes

B, S, H, F, E, TOP_K = 2, 2048, 1024, 3584, 8, 2
N_CORES = 8
P = 128
HK = H // P   # 8 contraction chunks for up-proj
FP = F // P   # 28 partition chunks of the FFN dim
HO = H // P   # 8 output-row chunks of H for the down-proj

BF16 = ml_dtypes.bfloat16

_BUILD_CACHE = {}
LAST_EXEC_TIME_NS = None


def _ensure_axon_hooks_stub():
    """bass_utils imports antenv.axon_hooks when BASS_TRACE is set; the
    agent image lacks it.  Register a None-hook stub so a stray
    BASS_TRACE env var degrades to an untraced run instead of crashing.
    """
    import sys, types

    try:
        import antenv.axon_hooks  # noqa: F401
        return
    except ImportError:
        pass
    mod = types.ModuleType("antenv.axon_hooks")
    mod._hook = None
    mod.set_axon_ntff_profile_hook = lambda h: setattr(mod, "_hook", h)
    mod.get_axon_ntff_profile_hook = lambda: mod._hook
    sys.modules["antenv.axon_hooks"] = mod
    try:
        import antenv

        antenv.axon_hooks = mod
    except ImportError:
        pass


def _chunks(total, maxc):
    """Split `total` into equal-ish chunks <= maxc (PSUM free-dim cap)."""
    n = -(-total // maxc)
    base, rem = divmod(total, n)
    sizes = [base + (1 if i < rem else 0) for i in range(n)]
    out, off = [], 0
    for c in sizes:
        out.append((off, c))
        off += c
    return out


def _build(C):
    """Build + compile the SPMD Bass program for token capacity C."""
    import concourse.bacc as bacc
    import concourse.mybir as mybir
    from concourse.tile import TileContext

    bf = mybir.dt.bfloat16
    f32 = mybir.dt.float32

    nc = bacc.Bacc("TRN2", target_bir_lowering=False, debug=False,
                   num_devices=N_CORES)
    # xe is packed [P, HK*C]: per-partition rows hold all HK contraction
    # chunks contiguously, so a multi-chunk column range is ONE wide DMA
    # (queues are descriptor-rate limited: any [128, w] piece costs
    # ~2.1us regardless of w, so fewer/wider pieces win).
    xe = nc.dram_tensor("xe", [P, HK * C], bf, kind="ExternalInput")
    w1p = nc.dram_tensor("w1p", [FP, P, H], bf, kind="ExternalInput")
    w3p = nc.dram_tensor("w3p", [FP, P, H], bf, kind="ExternalInput")
    w2p = nc.dram_tensor("w2p", [FP, P, H], bf, kind="ExternalInput")
    outT = nc.dram_tensor("outT", [HO, P, C], bf, kind="ExternalOutput")

    cn_chunks = _chunks(C, 512)
    NCI = len(cn_chunks)
    silu = mybir.ActivationFunctionType.Silu
    copy = mybir.ActivationFunctionType.Copy

    with TileContext(nc) as tc:
        with (
            tc.tile_pool(name="persist", bufs=1) as persist,
            tc.tile_pool(name="wload", bufs=3) as wload,
            tc.tile_pool(name="gpool", bufs=1) as gpool,
            tc.tile_pool(name="evac", bufs=4) as evac,
            tc.tile_pool(name="ost", bufs=3) as ost,
            tc.tile_pool(name="psum", bufs=4, space="PSUM") as psum,
        ):
            # HAM warmup: the PE clock-gate needs ~3.4us of sustained
            # activity to lift 1.2 -> 2.4 GHz, and the first DMA bytes
            # only land ~8.3us in (engine init + queue spin-up).  Dummy
            # matmuls bridge the gap; the memset runs on GpSimd.
            warm = persist.tile([P, 512], bf, tag="warm", name="warm")
            nc.gpsimd.memset(warm[:], 0.0)
            # 18 dummies (~420-760ns each -- same-bank groups don't
            # pipeline) bridge engine-start (~7.7us) to ~15.5us: the
            # startup set (xe 2.2MB + w1t0/w3t0) needs ~8us of ~400GB/s
            # aggregate DMA, so real chains can't run sooner anyway.
            # The PE never idles pre-ramp, keeping the HAM gate up.
            wps = psum.tile([P, 512], f32, tag="ps1", name="wps")
            for i in range(18):
                nc.tensor.matmul(wps[:], warm[:, 0:P], warm[:],
                                 start=True, stop=True)

            # Startup DMA schedule, ~133GB/s per queue (3-way HBM split).
            # The fp0 hk-outer chain starts as soon as w1t0/w3t0/xe[hk0]
            # land (~10.3us, still on the ramping clock) and consumes one
            # hk chunk per ~1us; each later piece lands just ahead of use.
            xet = persist.tile([P, HK * C], bf, tag="xe", name="xet")

            w1t0 = wload.tile([P, H], bf, tag="w1")
            nc.sync.dma_start(out=w1t0[:], in_=w1p[0])
            w3t0 = wload.tile([P, H], bf, tag="w3")
            nc.scalar.dma_start(out=w3t0[:], in_=w3p[0])
            nc.gpsimd.dma_start(out=xet[:, 0:2 * C], in_=xe[:, 0:2 * C])

            nc.sync.dma_start(out=xet[:, 2 * C:4 * C], in_=xe[:, 2 * C:4 * C])
            nc.scalar.dma_start(out=xet[:, 4 * C:6 * C],
                                in_=xe[:, 4 * C:6 * C])
            nc.gpsimd.dma_start(out=xet[:, 6 * C:8 * C],
                                in_=xe[:, 6 * C:8 * C])

            w1t1 = wload.tile([P, H], bf, tag="w1")
            nc.sync.dma_start(out=w1t1[:], in_=w1p[1])
            w3t1 = wload.tile([P, H], bf, tag="w3")
            nc.scalar.dma_start(out=w3t1[:], in_=w3p[1])

            # w2 residents stream on the gpsimd queue during phase A.
            w2t = []
            for fp in range(FP):
                t = persist.tile([P, H], bf, tag=f"w2_{fp}", name=f"w2_{fp}")
                nc.gpsimd.dma_start(out=t[:], in_=w2p[fp])
                w2t.append(t)

            gt = [gpool.tile([P, C], bf, tag=f"g{fp}", name=f"g{fp}")
                  for fp in range(FP)]

            # Phase A: h1T/h3T = w1/w3 @ xeT per 128-row chunk of F,
            # fused SwiGLU into gT (bf16).
            for fp in range(FP):
                if fp < 2:
                    # hk-outer: each matmul chain consumes xe[hk] as it
                    # lands instead of stalling on the whole activation
                    # load before the first instruction.  Two chains
                    # (~15us PE) cover the startup DMA window.
                    w1t, w3t = (w1t0, w3t0) if fp == 0 else (w1t1, w3t1)
                    pss = {}
                    for mat in (1, 3):
                        for ci in range(NCI):
                            pss[(mat, ci)] = psum.tile(
                                [P, 512], f32, tag=f"ps{mat}",
                                name=f"ps{mat}_c{ci}_f{fp}",
                            )
                    for hk in range(HK):
                        for mat, wt in ((1, w1t), (3, w3t)):
                            for ci, (coff, csz) in enumerate(cn_chunks):
                                nc.tensor.matmul(
                                    pss[(mat, ci)][:, :csz],
                                    wt[:, hk * P:(hk + 1) * P],
                                    xet[:, hk * C + coff:hk * C + coff + csz],
                                    start=(hk == 0), stop=(hk == HK - 1),
                                )
                    for ci, (coff, csz) in enumerate(cn_chunks):
                        sil = evac.tile([P, 512], f32, tag="sil",
                                        name=f"sil_f{fp}_{ci}")
                        nc.scalar.activation(
                            sil[:, :csz], pss[(1, ci)][:, :csz], silu)
                        nc.vector.tensor_mul(
                            gt[fp][:, coff:coff + csz], sil[:, :csz],
                            pss[(3, ci)][:, :csz],
                        )
                    continue
                w1t = wload.tile([P, H], bf, tag="w1")
                nc.sync.dma_start(out=w1t[:], in_=w1p[fp])
                w3t = wload.tile([P, H], bf, tag="w3")
                nc.scalar.dma_start(out=w3t[:], in_=w3p[fp])
                for (coff, csz) in cn_chunks:
                    ps1 = psum.tile([P, 512], f32, tag="ps1")
                    ps3 = psum.tile([P, 512], f32, tag="ps3")
                    # interleave the two chains mm-by-mm: half the chain
                    # boundaries, and ldweights always has a full matmul
                    # of slack to hide under
                    for hk in range(HK):
                        nc.tensor.matmul(
                            ps1[:, :csz],
                            w1t[:, hk * P:(hk + 1) * P],
                            xet[:, hk * C + coff:hk * C + coff + csz],
                            start=(hk == 0), stop=(hk == HK - 1),
                        )
                        nc.tensor.matmul(
                            ps3[:, :csz],
                            w3t[:, hk * P:(hk + 1) * P],
                            xet[:, hk * C + coff:hk * C + coff + csz],
                            start=(hk == 0), stop=(hk == HK - 1),
                        )
                    sil = evac.tile([P, 512], f32, tag="sil")
                    nc.scalar.activation(sil[:, :csz], ps1[:, :csz], silu)
                    nc.vector.tensor_mul(
                        gt[fp][:, coff:coff + csz], sil[:, :csz], ps3[:, :csz]
                    )

            # Phase B: outT[h] chunk [128 H-rows, csz tokens] =
            # sum_fp w2T-tile[fp,h] @ gT[fp].  Tokens ride the moving
            # axis, so the partial token chunk costs only its true
            # column count.  Each h stages into one full-width tile and
            # ships as a single large-packet DMA; routing weights are
            # applied host-side.  Shares the phase-A PSUM pool (no
            # pool-transition barrier).
            for h in range(HO):
                oh = ost.tile([P, C], bf, tag="o", name=f"o{h}")
                last_h = h == HO - 1
                for ci, (coff, csz) in enumerate(cn_chunks):
                    pb = psum.tile([P, 512], f32,
                                   tag="ps1" if (h * NCI + ci) % 2 == 0
                                   else "ps3")
                    for fp in range(FP):
                        nc.tensor.matmul(
                            pb[:, :csz],
                            w2t[fp][:, h * P:(h + 1) * P],
                            gt[fp][:, coff:coff + csz],
                            start=(fp == 0), stop=(fp == FP - 1),
                        )
                    nc.scalar.activation(oh[:, coff:coff + csz],
                                         pb[:, :csz], copy)
                    if last_h:
                        # ship the final h per-chunk as each evicts, so
                        # only the smallest piece drains after the last
                        # matmul (a [128,w] DMA costs ~2.1us for ANY w --
                        # never split; overlap instead)
                        e = nc.sync if ci % 2 == 0 else nc.scalar
                        e.dma_start(out=outT[h][:, coff:coff + csz],
                                    in_=oh[:, coff:coff + csz])
                if not last_h:
                    e = nc.sync if h % 2 == 0 else nc.scalar
                    e.dma_start(out=outT[h], in_=oh[:])

    nc.compile()
    return nc


def kernel(hidden_states, gate_w, w1, w2, w3, _trace=False):
    global LAST_EXEC_TIME_NS
    _ensure_axon_hooks_stub()
    from concourse.bass_utils import run_bass_kernel_spmd

    x = np.asarray(hidden_states, dtype=np.float32).reshape(-1, H)
    gate_w = np.asarray(gate_w, dtype=np.float32)
    w1 = np.asarray(w1, dtype=np.float32)
    w2 = np.asarray(w2, dtype=np.float32)
    w3 = np.asarray(w3, dtype=np.float32)
    T = x.shape[0]

    # Router (f32, same math as the module): softmax over experts, top-2,
    # renormalized weights.
    logits = x @ gate_w.T
    p = np.exp(logits - logits.max(-1, keepdims=True))
    p /= p.sum(-1, keepdims=True)
    sel = np.argpartition(-p, TOP_K - 1, axis=-1)[:, :TOP_K]
    rw = np.take_along_axis(p, sel, axis=-1)
    rw = rw / rw.sum(-1, keepdims=True)

    idx_e, cv_e = [], []
    for e in range(E):
        hit = sel == e                      # [T, K]
        idx = np.nonzero(hit.any(axis=1))[0]
        w = np.where(hit[idx, 0], rw[idx, 0], rw[idx, 1])
        idx_e.append(idx)
        cv_e.append(w.astype(np.float32))

    # SBUF budget (xe + gT residents) caps the per-run token capacity.
    # Actual data peaks at cmax ~1071; the segment loop only engages for
    # pathologically imbalanced routing.
    CMAX_HW = 1344
    cmax = max(len(i) for i in idx_e)
    n_seg = max(1, -(-cmax // CMAX_HW))
    seg_idx = [np.array_split(idx_e[e], n_seg) for e in range(E)]
    seg_cv = [np.array_split(cv_e[e], n_seg) for e in range(E)]
    C = max(512, max(len(s) for parts in seg_idx for s in parts))

    if C not in _BUILD_CACHE:
        _BUILD_CACHE[C] = _build(C)
    nc = _BUILD_CACHE[C]

    x_bf = x.astype(BF16)
    w_packed = []
    for e in range(E):
        w1pk = np.ascontiguousarray(
            w1[e].astype(BF16).reshape(FP, P, HK, P).transpose(0, 3, 2, 1)
        ).reshape(FP, P, H)
        w3pk = np.ascontiguousarray(
            w3[e].astype(BF16).reshape(FP, P, HK, P).transpose(0, 3, 2, 1)
        ).reshape(FP, P, H)
        w2pk = np.ascontiguousarray(w2[e].T.astype(BF16)).reshape(FP, P, H)
        w_packed.append((w1pk, w3pk, w2pk))

    out = np.zeros((T, H), dtype=np.float32)
    LAST_EXEC_TIME_NS = None
    for seg in range(n_seg):
        in_maps = []
        for e in range(E):
            idx = seg_idx[e][seg]
            n = len(idx)
            xeT = np.zeros((H, C), dtype=BF16)
            xeT[:, :n] = x_bf[idx].T
            w1pk, w3pk, w2pk = w_packed[e]
            # pack [P, HK*C]: partition-major rows holding all HK chunks
            xpk = np.ascontiguousarray(
                xeT.reshape(HK, P, C).transpose(1, 0, 2).reshape(P, HK * C))
            in_maps.append({
                "xe": xpk,
                "w1p": w1pk,
                "w3p": w3pk,
                "w2p": w2pk,
            })
        res = run_bass_kernel_spmd(
            nc, in_maps, core_ids=list(range(N_CORES)), trace=_trace
        )
        if res.exec_time_ns is not None:
            LAST_EXEC_TIME_NS = (LAST_EXEC_TIME_NS or 0) + res.exec_time_ns
        for e in range(E):
            idx = seg_idx[e][seg]
            n = len(idx)
            if n:
                oT = np.asarray(res.results[e]["outT"],
                                dtype=np.float32).reshape(H, C)
                out[idx] += oT[:, :n].T * seg_cv[e][seg][:, None]
    return out.reshape(B, S, H)


# revision 23
# speedup vs baseline: 1.0076x; 1.0005x over previous
"""Mixtral-style MoE (top-2 of 8 experts) on 8 TRN2 NeuronCores.

Strategy (expert-parallel, matching TENSOR_EXPERT_PARALLEL):
  - Host: router (logits -> softmax -> top-2 -> normalized weights), then
    shard: core e receives the tokens routed to expert e (gathered and
    pre-transposed to [H, C]) plus expert e's w1/w3/w2 (bf16, pre-packed
    into PE-friendly [128 x free] tiles).
  - Device (SPMD, identical program on 8 cores): h1T = w1 @ xeT,
    h3T = w3 @ xeT, gT = silu(h1T) * h3T (bf16), then the down-proj in
    output-transposed orientation: outT[h, :] = sum_f w2T-tile @ gT
    (tokens stay on the moving axis, so no padded-partition waste on
    the partial token chunk).  Pure GEMM pipeline; all DMAs linear.
  - Host: scatter-add each core's [H, count_e] contribution (scaled by
    the routing weight, applied host-side) into the [T, H] output.

Compute in bf16 (fp32 PSUM accumulation) keeps the TensorEngine at its
78.6 TF/s peak; fp8 DoubleRow would be ~1.8x faster but its ~3-6%
quantization error blows the 2e-2 correctness budget (measured).
Sparse routing means each core does C = max expert load (~1071)
token-columns instead of all 4096.

PE-time floor at C=1071: phase A 28*2*8*C = 200us + phase B 8*28*C =
100us = 300us @2.4GHz.  Engine init (~6.6us) and first DMA bytes
(~8.3us) are fixed NEFF costs; warmup matmuls bridge them while the
HAM clock ramps.  fp0+fp1 run hk-outer *interleaved* with the token
range split {ci0,ci1} then {ci2} (exactly 8 PSUM banks), stretching
the xe consumption window so three ~130GB/s DMA queues deliver every
chunk just in time -- no stalls, no half-clock dip.  Phase B stages
each 128-row H chunk in a full-width SBUF tile and ships it as one
large-packet DMA, alternating queues, so the output drain collapses
to the final piece plus teardown.
"""

import numpy as np
import ml_dtypes

B, S, H, F, E, TOP_K = 2, 2048, 1024, 3584, 8, 2
N_CORES = 8
P = 128
HK = H // P   # 8 contraction chunks for up-proj
FP = F // P   # 28 partition chunks of the FFN dim
HO = H // P   # 8 output-row chunks of H for the down-proj

BF16 = ml_dtypes.bfloat16

_BUILD_CACHE = {}
LAST_EXEC_TIME_NS = None


def _ensure_axon_hooks_stub():
    """bass_utils imports antenv.axon_hooks when BASS_TRACE is set; the
    agent image lacks it.  Register a None-hook stub so a stray
    BASS_TRACE env var degrades to an untraced run instead of crashing.
    """
    import sys, types

    try:
        import antenv.axon_hooks  # noqa: F401
        return
    except ImportError:
        pass
    mod = types.ModuleType("antenv.axon_hooks")
    mod._hook = None
    mod.set_axon_ntff_profile_hook = lambda h: setattr(mod, "_hook", h)
    mod.get_axon_ntff_profile_hook = lambda: mod._hook
    sys.modules["antenv.axon_hooks"] = mod
    try:
        import antenv

        antenv.axon_hooks = mod
    except ImportError:
        pass


def _chunks(total, maxc):
    """Split `total` into equal-ish chunks <= maxc (PSUM free-dim cap)."""
    n = -(-total // maxc)
    base, rem = divmod(total, n)
    sizes = [base + (1 if i < rem else 0) for i in range(n)]
    out, off = [], 0
    for c in sizes:
        out.append((off, c))
        off += c
    return out


def _build(C):
    """Build + compile the SPMD Bass program for token capacity C."""
    import concourse.bacc as bacc
    import concourse.mybir as mybir
    from concourse.tile import TileContext

    bf = mybir.dt.bfloat16
    f32 = mybir.dt.float32

    nc = bacc.Bacc("TRN2", target_bir_lowering=False, debug=False,
                   num_devices=N_CORES)
    # xe is packed [P, HK*C]: per-partition rows hold all HK contraction
    # chunks contiguously, so a multi-chunk column range is ONE wide DMA
    # (queues are descriptor-rate limited: any [128, w] piece costs
    # ~2.1us regardless of w, so fewer/wider pieces win).
    xe = nc.dram_tensor("xe", [P, HK * C], bf, kind="ExternalInput")
    w1p = nc.dram_tensor("w1p", [FP, P, H], bf, kind="ExternalInput")
    w3p = nc.dram_tensor("w3p", [FP, P, H], bf, kind="ExternalInput")
    w2p = nc.dram_tensor("w2p", [FP, P, H], bf, kind="ExternalInput")
    outT = nc.dram_tensor("outT", [HO, P, C], bf, kind="ExternalOutput")

    cn_chunks = _chunks(C, 512)
    NCI = len(cn_chunks)
    silu = mybir.ActivationFunctionType.Silu
    copy = mybir.ActivationFunctionType.Copy

    with TileContext(nc) as tc:
        with (
            tc.tile_pool(name="persist", bufs=1) as persist,
            tc.tile_pool(name="wload", bufs=3) as wload,
            tc.tile_pool(name="gpool", bufs=1) as gpool,
            tc.tile_pool(name="evac", bufs=4) as evac,
            tc.tile_pool(name="ost", bufs=3) as ost,
            tc.tile_pool(name="psum", bufs=4, space="PSUM") as psum,
        ):
            # HAM warmup: the PE clock-gate needs ~3.4us of sustained
            # activity to lift 1.2 -> 2.4 GHz, and the first DMA bytes
            # only land ~8.3us in (engine init + queue spin-up).  Dummy
            # matmuls bridge the gap; the memset runs on GpSimd.
            warm = persist.tile([P, 512], bf, tag="warm", name="warm")
            nc.gpsimd.memset(warm[:], 0.0)
            # 18 dummies (~420-760ns each -- same-bank groups don't
            # pipeline) bridge engine-start (~7.7us) to ~15.5us: the
            # startup set (xe 2.2MB + w1t0/w3t0) needs ~8us of ~400GB/s
            # aggregate DMA, so real chains can't run sooner anyway.
            # The PE never idles pre-ramp, keeping the HAM gate up.
            wps = psum.tile([P, 512], f32, tag="ps1", name="wps")
            for i in range(18):
                nc.tensor.matmul(wps[:], warm[:, 0:P], warm[:],
                                 start=True, stop=True)

            # Startup DMA schedule, ~133GB/s per queue (3-way HBM split).
            # The fp0 hk-outer chain starts as soon as w1t0/w3t0/xe[hk0]
            # land (~10.3us, still on the ramping clock) and consumes one
            # hk chunk per ~1us; each later piece lands just ahead of use.
            xet = persist.tile([P, HK * C], bf, tag="xe", name="xet")

            w1t0 = wload.tile([P, H], bf, tag="w1")
            nc.sync.dma_start(out=w1t0[:], in_=w1p[0])
            w3t0 = wload.tile([P, H], bf, tag="w3")
            nc.scalar.dma_start(out=w3t0[:], in_=w3p[0])
            nc.gpsimd.dma_start(out=xet[:, 0:2 * C], in_=xe[:, 0:2 * C])

            nc.sync.dma_start(out=xet[:, 2 * C:4 * C], in_=xe[:, 2 * C:4 * C])
            nc.scalar.dma_start(out=xet[:, 4 * C:6 * C],
                                in_=xe[:, 4 * C:6 * C])
            nc.gpsimd.dma_start(out=xet[:, 6 * C:8 * C],
                                in_=xe[:, 6 * C:8 * C])

            w1t1 = wload.tile([P, H], bf, tag="w1")
            nc.sync.dma_start(out=w1t1[:], in_=w1p[1])
            w3t1 = wload.tile([P, H], bf, tag="w3")
            nc.scalar.dma_start(out=w3t1[:], in_=w3p[1])

            # w2 residents stream on the gpsimd queue during phase A.
            w2t = []
            for fp in range(FP):
                t = persist.tile([P, H], bf, tag=f"w2_{fp}", name=f"w2_{fp}")
                nc.gpsimd.dma_start(out=t[:], in_=w2p[fp])
                w2t.append(t)

            gt = [gpool.tile([P, C], bf, tag=f"g{fp}", name=f"g{fp}")
                  for fp in range(FP)]

            # Phase A: h1T/h3T = w1/w3 @ xeT per 128-row chunk of F,
            # fused SwiGLU into gT (bf16).
            for fp in range(FP):
                if fp < 2:
                    # hk-outer: each matmul chain consumes xe[hk] as it
                    # lands instead of stalling on the whole activation
                    # load before the first instruction.  Two chains
                    # (~15us PE) cover the startup DMA window.
                    w1t, w3t = (w1t0, w3t0) if fp == 0 else (w1t1, w3t1)
                    pss = {}
                    for mat in (1, 3):
                        for ci in range(NCI):
                            pss[(mat, ci)] = psum.tile(
                                [P, 512], f32, tag=f"ps{mat}",
                                name=f"ps{mat}_c{ci}_f{fp}",
                            )
                    for hk in range(HK):
                        for mat, wt in ((1, w1t), (3, w3t)):
                            for ci, (coff, csz) in enumerate(cn_chunks):
                                nc.tensor.matmul(
                                    pss[(mat, ci)][:, :csz],
                                    wt[:, hk * P:(hk + 1) * P],
                                    xet[:, hk * C + coff:hk * C + coff + csz],
                                    start=(hk == 0), stop=(hk == HK - 1),
                                )
                    for ci, (coff, csz) in enumerate(cn_chunks):
                        sil = evac.tile([P, 512], f32, tag="sil",
                                        name=f"sil_f{fp}_{ci}")
                        nc.scalar.activation(
                            sil[:, :csz], pss[(1, ci)][:, :csz], silu)
                        nc.vector.tensor_mul(
                            gt[fp][:, coff:coff + csz], sil[:, :csz],
                            pss[(3, ci)][:, :csz],
                        )
                    continue
                # both streams on sync: the scalar engine's sequencer is
                # needed for evictions, and one queue sustains the 72GB/s
                # steady-state weight pace easily
                w1t = wload.tile([P, H], bf, tag="w1")
                nc.sync.dma_start(out=w1t[:], in_=w1p[fp])
                w3t = wload.tile([P, H], bf, tag="w3")
                nc.sync.dma_start(out=w3t[:], in_=w3p[fp])
                for (coff, csz) in cn_chunks:
                    ps1 = psum.tile([P, 512], f32, tag="ps1")
                    ps3 = psum.tile([P, 512], f32, tag="ps3")
                    # interleave the two chains mm-by-mm: half the chain
                    # boundaries, and ldweights always has a full matmul
                    # of slack to hide under
                    for hk in range(HK):
                        nc.tensor.matmul(
                            ps1[:, :csz],
                            w1t[:, hk * P:(hk + 1) * P],
                            xet[:, hk * C + coff:hk * C + coff + csz],
                            start=(hk == 0), stop=(hk == HK - 1),
                        )
                        nc.tensor.matmul(
                            ps3[:, :csz],
                            w3t[:, hk * P:(hk + 1) * P],
                            xet[:, hk * C + coff:hk * C + coff + csz],
                            start=(hk == 0), stop=(hk == HK - 1),
                        )
                    sil = evac.tile([P, 512], f32, tag="sil")
                    nc.scalar.activation(sil[:, :csz], ps1[:, :csz], silu)
                    nc.vector.tensor_mul(
                        gt[fp][:, coff:coff + csz], sil[:, :csz], ps3[:, :csz]
                    )

            # Phase B: outT[h] chunk [128 H-rows, csz tokens] =
            # sum_fp w2T-tile[fp,h] @ gT[fp].  Tokens ride the moving
            # axis, so the partial token chunk costs only its true
            # column count.  Each h stages into one full-width tile and
            # ships as a single large-packet DMA; routing weights are
            # applied host-side.  Shares the phase-A PSUM pool (no
            # pool-transition barrier).
            for h in range(HO):
                oh = ost.tile([P, C], bf, tag="o", name=f"o{h}")
                last_h = h == HO - 1
                for ci, (coff, csz) in enumerate(cn_chunks):
                    pb = psum.tile([P, 512], f32,
                                   tag="ps1" if (h * NCI + ci) % 2 == 0
                                   else "ps3")
                    for fp in range(FP):
                        nc.tensor.matmul(
                            pb[:, :csz],
                            w2t[fp][:, h * P:(h + 1) * P],
                            gt[fp][:, coff:coff + csz],
                            start=(fp == 0), stop=(fp == FP - 1),
                        )
                    nc.scalar.activation(oh[:, coff:coff + csz],
                                         pb[:, :csz], copy)
                    if last_h:
                        # ship the final h per-chunk as each evicts, so
                        # only the smallest piece drains after the last
                        # matmul (a [128,w] DMA costs ~2.1us for ANY w --
                        # never split; overlap instead)
                        e = nc.sync if ci % 2 == 0 else nc.scalar
                        e.dma_start(out=outT[h][:, coff:coff + csz],
                                    in_=oh[:, coff:coff + csz])
                if not last_h:
                    e = nc.sync if h % 2 == 0 else nc.scalar
                    e.dma_start(out=outT[h], in_=oh[:])

    nc.compile()
    return nc


def kernel(hidden_states, gate_w, w1, w2, w3, _trace=False):
    global LAST_EXEC_TIME_NS
    _ensure_axon_hooks_stub()
    from concourse.bass_utils import run_bass_kernel_spmd

    x = np.asarray(hidden_states, dtype=np.float32).reshape(-1, H)
    gate_w = np.asarray(gate_w, dtype=np.float32)
    w1 = np.asarray(w1, dtype=np.float32)
    w2 = np.asarray(w2, dtype=np.float32)
    w3 = np.asarray(w3, dtype=np.float32)
    T = x.shape[0]

    # Router (f32, same math as the module): softmax over experts, top-2,
    # renormalized weights.
    logits = x @ gate_w.T
    p = np.exp(logits - logits.max(-1, keepdims=True))
    p /= p.sum(-1, keepdims=True)
    sel = np.argpartition(-p, TOP_K - 1, axis=-1)[:, :TOP_K]
    rw = np.take_along_axis(p, sel, axis=-1)
    rw = rw / rw.sum(-1, keepdims=True)

    idx_e, cv_e = [], []
    for e in range(E):
        hit = sel == e                      # [T, K]
        idx = np.nonzero(hit.any(axis=1))[0]
        w = np.where(hit[idx, 0], rw[idx, 0], rw[idx, 1])
        idx_e.append(idx)
        cv_e.append(w.astype(np.float32))

    # SBUF budget (xe + gT residents) caps the per-run token capacity.
    # Actual data peaks at cmax ~1071; the segment loop only engages for
    # pathologically imbalanced routing.
    CMAX_HW = 1344
    cmax = max(len(i) for i in idx_e)
    n_seg = max(1, -(-cmax // CMAX_HW))
    seg_idx = [np.array_split(idx_e[e], n_seg) for e in range(E)]
    seg_cv = [np.array_split(cv_e[e], n_seg) for e in range(E)]
    C = max(512, max(len(s) for parts in seg_idx for s in parts))

    if C not in _BUILD_CACHE:
        _BUILD_CACHE[C] = _build(C)
    nc = _BUILD_CACHE[C]

    x_bf = x.astype(BF16)
    w_packed = []
    for e in range(E):
        w1pk = np.ascontiguousarray(
            w1[e].astype(BF16).reshape(FP, P, HK, P).transpose(0, 3, 2, 1)
        ).reshape(FP, P, H)
        w3pk = np.ascontiguousarray(
            w3[e].astype(BF16).reshape(FP, P, HK, P).transpose(0, 3, 2, 1)
        ).reshape(FP, P, H)
        w2pk = np.ascontiguousarray(w2[e].T.astype(BF16)).reshape(FP, P, H)
        w_packed.append((w1pk, w3pk, w2pk))

    out = np.zeros((T, H), dtype=np.float32)
    LAST_EXEC_TIME_NS = None
    for seg in range(n_seg):
        in_maps = []
        for e in range(E):
            idx = seg_idx[e][seg]
            n = len(idx)
            xeT = np.zeros((H, C), dtype=BF16)
            xeT[:, :n] = x_bf[idx].T
            w1pk, w3pk, w2pk = w_packed[e]
            # pack [P, HK*C]: partition-major rows holding all HK chunks
            xpk = np.ascontiguousarray(
                xeT.reshape(HK, P, C).transpose(1, 0, 2).reshape(P, HK * C))
            in_maps.append({
                "xe": xpk,
                "w1p": w1pk,
                "w3p": w3pk,
                "w2p": w2pk,
            })
        res = run_bass_kernel_spmd(
            nc, in_maps, core_ids=list(range(N_CORES)), trace=_trace
        )
        if res.exec_time_ns is not None:
            LAST_EXEC_TIME_NS = (LAST_EXEC_TIME_NS or 0) + res.exec_time_ns
        for e in range(E):
            idx = seg_idx[e][seg]
            n = len(idx)
            if n:
                oT = np.asarray(res.results[e]["outT"],
                                dtype=np.float32).reshape(H, C)
                out[idx] += oT[:, :n].T * seg_cv[e][seg][:, None]
    return out.reshape(B, S, H)
